# revision 47
# baseline (speedup 1.0000x reference)
"""Trainium2 Bass kernel for nn_CrossAggregator (gnn_message_passing).

out[g,o] = self[g]·W1[o,:] + ea_g^T A_o eb_g,  g=(b,m), A_o = W[o,128:].reshape(128,128)
ea/eb = masked means over 32 neighbors (t=0 / t=1).

Design v3 (per core, batch/8 data-parallel, G=512 rows), all heavy data bf16:
- single DMA queue (sync), strict priority order: consts, nb, na, W2 —
  chunks land in-place in full-resident tiles (no pool-recycle stalls).
- eb-side: masked-mean via 32 bf16 matmuls with a banded selector (BIG) as
  stationary -> ebT [j,g] in PSUM; Act-copied to SBUF bf16; 3 partition
  rotations ebT[(p+32r)%128, g] via PE permutation matmuls + Act copies
  -> ebTall [128, 4G].
- ea-side: ONE matmul per slab with a block-diagonal ones/32 stationary (BD)
  broadcasts all 4 i's of the slab into 4 partition bands -> repQ [128,G]
  (32 rep passes instead of 128).
- pt: repQ PSUM -> Act copy to bf16 -> one DVE 2x multiply per slab against
  the 4 ebT rotations (in0 dense, in1 stride-0 broadcast) -> pth [128, 4G].
- main contraction: 4 matmuls per slab, stationary W2stat[ig,r][p,o] =
  W2[o, 4ig+p//32, (p+32r)%128]; the rotation is folded into host W2
  packing so all (i,j) pairs are covered exactly once.
  PE total: 32 eb + 3 rot + 32 rep + 128 main + 1 W1 = 196 passes.
- host does only layout transforms (shard/permute/pack/bf16 cast) + out
  transpose + bias add.
"""
import sys
import numpy as np

for _p in ("/opt/trn_rl_repo", "/root/.axon_site/_ro/trn_rl_repo"):
    if _p not in sys.path:
        sys.path.insert(0, _p)

B, M, TWO, NN, D = 1024, 4, 2, 32, 128
OUT = 128
NCORES = 8
BC = B // NCORES          # batches per core
G = BC * M                # 512 rows per core
NIG = D // 4              # 32 slabs of 4 features (partition packing (q,n))
CH = 8                    # slabs per DMA chunk -> 4 chunks per side
NCHUNK = NIG // CH

_CACHE = {}


def _build_nc():
    import os
    import concourse.bacc as bacc_mod
    import concourse.mybir as mybir
    from concourse.tile import TileContext

    F32 = mybir.dt.float32
    BF16 = mybir.dt.bfloat16
    MUL = mybir.AluOpType.mult

    nc = bacc_mod.Bacc(None)

    d_naA = nc.declare_dram_parameter("naA", [128, NIG * G], BF16, isOutput=False)
    d_nbA = nc.declare_dram_parameter("nbA", [128, NIG * G], BF16, isOutput=False)
    d_maskA = nc.declare_dram_parameter("maskA", [128, G], BF16, isOutput=False)
    d_maskB = nc.declare_dram_parameter("maskB", [128, G], BF16, isOutput=False)
    d_selfT = nc.declare_dram_parameter("selfT", [D, G], BF16, isOutput=False)
    d_W1 = nc.declare_dram_parameter("W1a", [D, OUT], BF16, isOutput=False)
    d_W2 = nc.declare_dram_parameter("W2A", [D, NIG * 4 * OUT], BF16, isOutput=False)
    d_BIG = nc.declare_dram_parameter("BIG", [128, 252], BF16, isOutput=False)
    d_BD = nc.declare_dram_parameter("BD", [128, 128], BF16, isOutput=False)
    d_PROT = nc.declare_dram_parameter("PROT", [128, 3 * 128], BF16, isOutput=False)
    d_out = nc.declare_dram_parameter("outT", [OUT, G], F32, isOutput=True)

    NDVE = int(os.environ.get("PT_NDVE", "4"))
    LOOK = int(os.environ.get("REP_LOOK", "2"))
    REP_BUFS = int(os.environ.get("REP_BUFS", "4"))
    EBT_BUFS = int(os.environ.get("EBT_BUFS", "2"))

    with TileContext(nc) as tc:
        with (
            tc.tile_pool(name="const", bufs=1) as cpool,
            tc.tile_pool(name="big", bufs=1) as bigpool,
            tc.tile_pool(name="rq", bufs=3) as rqpool,
            tc.tile_pool(name="pt", bufs=3) as ptpool,
            tc.tile_pool(name="misc", bufs=1) as mpool,
            tc.tile_pool(name="ps_ebt", bufs=EBT_BUFS, space="PSUM") as ps_ebt,
            tc.tile_pool(name="ps_rep", bufs=REP_BUFS, space="PSUM") as ps_rep,
            tc.tile_pool(name="ps_out", bufs=1, space="PSUM") as ps_out,
        ):
            # maskB gates the very first mb mask multiply; everything else
            # can trail the first nb chunk (each DMA issue costs ~0.6us on
            # the sync engine, so nb0 goes as early as possible).
            maskB_t = cpool.tile([128, G], BF16, tag="mb")
            nc.sync.dma_start(out=maskB_t[:], in_=d_maskB[:])

            # full-resident buffers; nb/na land in-place, masks applied
            # in-place (read-before-write on the streaming DVE is safe)
            mb_full = bigpool.tile([128, NIG * G], BF16, tag="mbF")
            ma_full = bigpool.tile([128, NIG * G], BF16, tag="maF")
            w2_full = bigpool.tile([128, NIG * 4 * OUT], BF16, tag="w2F")
            # ebTall holds the 4 rotations twice ([0:4G] and [4G:8G]) so a
            # paired (2-slab) pth op can read a fully dense in0
            ebTall = bigpool.tile([128, 8 * G], BF16, tag="ebAll")

            # heavy DMAs in priority order on the sync queue: nb first
            # (gates the serial eb chain), tapered chunks so the last slabs
            # land with minimal tail; then na0, the rest of the consts,
            # W2 chunk 0 (gates first main matmuls), then the rest.
            NB_CHUNKS = [(0, 2), (2, 6), (8, 4), (12, 4), (16, 4), (20, 4),
                         (24, 4), (28, 2), (30, 1), (31, 1)]
            big_t = cpool.tile([128, 252], BF16, tag="big")
            prot_t = cpool.tile([128, 3 * 128], BF16, tag="prot")
            maskA_t = cpool.tile([128, G], BF16, tag="ma")
            bd_t = cpool.tile([128, 128], BF16, tag="bd")

            def na_dma(s0, sl):
                nc.sync.dma_start(
                    out=ma_full[:, s0 * G : (s0 + sl) * G],
                    in_=d_naA[:, s0 * G : (s0 + sl) * G],
                )

            for ci, (s0, sl) in enumerate(NB_CHUNKS):
                nc.sync.dma_start(
                    out=mb_full[:, s0 * G : (s0 + sl) * G],
                    in_=d_nbA[:, s0 * G : (s0 + sl) * G],
                )
                if ci == 0:
                    # BIG gates the first eb matmul (~1us after mb0 lands)
                    nc.sync.dma_start(out=big_t[:], in_=d_BIG[:])
                elif ci == 1:
                    # PROT gates the rotation matmuls (~12us later)
                    nc.sync.dma_start(out=prot_t[:], in_=d_PROT[:])
            def w2_dma(ig0, nig):
                nc.sync.dma_start(
                    out=w2_full[:, ig0 * 4 * OUT : (ig0 + nig) * 4 * OUT],
                    in_=d_W2[:, ig0 * 4 * OUT : (ig0 + nig) * 4 * OUT],
                )

            nc.sync.dma_start(out=maskA_t[:], in_=d_maskA[:])
            nc.sync.dma_start(out=bd_t[:], in_=d_BD[:])
            na_dma(0, 4)
            w2_dma(0, 2)
            na_dma(4, 4)
            selfT_t = cpool.tile([D, G], BF16, tag="sT")
            nc.sync.dma_start(out=selfT_t[:], in_=d_selfT[:])
            w1_t = cpool.tile([D, OUT], BF16, tag="w1")
            nc.sync.dma_start(out=w1_t[:], in_=d_W1[:])
            w2_dma(2, 6)
            na_dma(8, 8)
            w2_dma(8, 8)
            na_dma(16, 8)
            w2_dma(16, 8)
            na_dma(24, 8)
            w2_dma(24, 8)

            def emit_mask(full_t, mask_t, s0, sl, eng=None):
                (eng or nc.vector).tensor_tensor(
                    out=full_t[:, s0 * G : (s0 + sl) * G].rearrange(
                        "p (s c) -> p s c", s=sl
                    ),
                    in0=full_t[:, s0 * G : (s0 + sl) * G].rearrange(
                        "p (s c) -> p s c", s=sl
                    ),
                    in1=mask_t[:][:, None, :].broadcast_to([128, sl, G]),
                    op=MUL,
                )

            # ---- EB phase: ebT[j, g] in PSUM via banded-selector matmuls ----
            p_ebt = ps_ebt.tile([128, G], F32, tag="ebt")
            for ci, (s0, sl) in enumerate(NB_CHUNKS):
                emit_mask(mb_full, maskB_t, s0, sl)
                for u in range(sl):
                    jg = s0 + u
                    nc.tensor.matmul(
                        p_ebt[:],
                        big_t[:, 124 - 4 * jg : 252 - 4 * jg],
                        mb_full[:, jg * G : (jg + 1) * G],
                        start=(jg == 0),
                        stop=(jg == NIG - 1),
                    )


            # ma[0:4] right behind the mb-mask tail (na[0:4] lands just
            # after the nb stream); gates reps 0-3
            emit_mask(ma_full, maskA_t, 0, 4)

            # ebT -> SBUF bf16 (rotation 0); rotations 1-3 via PE permutation
            # matmuls (PE is otherwise idle here) + Act copies; then one DVE
            # 4x copy duplicates [0:4G] -> [4G:8G] for the paired pth in0.
            nc.scalar.copy(out=ebTall[:, 0:G], in_=p_ebt[:])
            for r in range(1, 4):
                p_rot = ps_ebt.tile([128, G], F32, tag="ebt")
                nc.tensor.matmul(
                    p_rot[:],
                    prot_t[:, (r - 1) * 128 : r * 128],
                    ebTall[:, 0:G],
                    start=True,
                    stop=True,
                )
                nc.scalar.copy(
                    out=ebTall[:, r * G : (r + 1) * G], in_=p_rot[:]
                )


            # ---- MAIN phase ----
            p_out = ps_out.tile([OUT, G], F32, tag="out")
            nc.tensor.matmul(p_out[:], w1_t[:], selfT_t[:], start=True, stop=False)

            rep_tiles = {}
            rq_tiles = {}
            pth_tiles = {}
            NPAIR = NIG // 2

            def emit_rep(ig):
                rep = ps_rep.tile([128, G], F32, tag="rep")
                nc.tensor.matmul(
                    rep[:],
                    bd_t[:],
                    ma_full[:, ig * G : (ig + 1) * G],
                    start=True,
                    stop=True,
                )
                rep_tiles[ig] = rep

            def emit_cp(k):
                # rq pair tile: halves written by two Act copies
                rq = rqpool.tile([128, 2 * G], BF16, tag="rq")
                for u in range(2):
                    rep = rep_tiles.pop(2 * k + u)
                    nc.scalar.copy(out=rq[:, u * G : (u + 1) * G], in_=rep[:])
                rq_tiles[k] = rq

            def emit_pth(k, split=False):
                # one DVE op per slab PAIR: out/in0 fully dense 4D, in1
                # broadcasts each slab's rq across the 4 rotations.
                # split=True: two 4G ops reading ebTall[0:4G] only — used
                # for pair 0 (before the dup lands) and the last pair
                # (first 4 main matmuls start half an op earlier).
                rq = rq_tiles.pop(k)
                pth = ptpool.tile([128, 8 * G], BF16, tag="pth")
                if split:
                    for u in range(2):
                        nc.vector.tensor_tensor(
                            out=pth[:, u * 4 * G : (u + 1) * 4 * G].rearrange(
                                "p (r c) -> p r c", r=4
                            ),
                            in0=ebTall[:, 0 : 4 * G].rearrange(
                                "p (r c) -> p r c", r=4
                            ),
                            in1=rq[:, u * G : (u + 1) * G][:, None, :]
                            .broadcast_to([128, 4, G]),
                            op=MUL,
                        )
                else:
                    nc.vector.tensor_tensor(
                        out=pth[:].rearrange("p (i r c) -> p i r c", i=2, r=4),
                        in0=ebTall[:].rearrange("p (i r c) -> p i r c", i=2, r=4),
                        in1=rq[:].rearrange("p (i c) -> p i c", i=2)[:, :, None, :]
                        .broadcast_to([128, 2, 4, G]),
                        op=MUL,
                    )
                pth_tiles[k] = pth

            for ig in range(2 * min(LOOK, NPAIR)):
                emit_rep(ig)
            emit_cp(0)
            emit_pth(0, split=True)
            # duplicate ebTall[0:4G] -> [4G:8G] for the paired in0 (4x DVE
            # copy, off the pth0 critical path)
            nc.vector.tensor_scalar_mul(
                ebTall[:, 4 * G : 8 * G], ebTall[:, 0 : 4 * G], 1.0
            )
            # remaining na masks threaded through the pair loop in 8/4-slab
            # pieces, each well ahead of its rep deadline (rep(2k+2*LOOK+1)
            # is emitted at pair k).
            MA_SCHED = {0: (4, 4), 1: (8, 8), 2: (16, 8), 4: (24, 8)}
            for k in range(NPAIR):
                if k in MA_SCHED:
                    s0, sl = MA_SCHED[k]
                    emit_mask(ma_full, maskA_t, s0, sl)
                if k + LOOK < NPAIR:
                    emit_rep(2 * k + 2 * LOOK)
                    emit_rep(2 * k + 2 * LOOK + 1)
                if k + 1 < NPAIR:
                    emit_cp(k + 1)
                    emit_pth(k + 1, split=(k + 1 == NPAIR - 1))
                pth = pth_tiles.pop(k)
                for u in range(2):
                    ig = 2 * k + u
                    for r in range(4):
                        nc.tensor.matmul(
                            p_out[:],
                            w2_full[:, (ig * 4 + r) * OUT : (ig * 4 + r + 1) * OUT],
                            pth[:, (u * 4 + r) * G : (u * 4 + r + 1) * G],
                            start=False,
                            stop=(ig == NIG - 1 and r == 3),
                        )

            out_sb = mpool.tile([OUT, G], F32, tag="osb")
            nc.scalar.copy(out=out_sb[:, 0 : G // 2], in_=p_out[:, 0 : G // 2])
            nc.sync.dma_start(out=d_out[:, 0 : G // 2], in_=out_sb[:, 0 : G // 2])
            nc.scalar.copy(out=out_sb[:, G // 2 :], in_=p_out[:, G // 2 :])
            nc.sync.dma_start(out=d_out[:, G // 2 :], in_=out_sb[:, G // 2 :])

    nc.finalize()
    return nc


def _host_prep(self_vectors, neighbor_vectors, masks, W):
    import ml_dtypes

    f32 = np.float32
    bf16 = ml_dtypes.bfloat16
    sv = np.asarray(self_vectors, dtype=f32)
    nv = np.asarray(neighbor_vectors, dtype=f32)
    mk = np.asarray(masks, dtype=f32)
    Wf = np.asarray(W, dtype=f32)

    # per-core packs: partition p = (q, n) holds feature j = 4*ig + q
    # cols = (ig, g)
    nvc = nv.reshape(NCORES, G, TWO, NN, D)          # [c, g, t, n, d]

    def pack_side(t):
        arr = nvc[:, :, t]                            # [c, g, n, d]
        arr = arr.transpose(0, 3, 2, 1)               # [c, d, n, g]
        arr = arr.reshape(NCORES, NIG, 4, NN, G)      # [c, ig, q, n, g]
        arr = arr.transpose(0, 2, 3, 1, 4)            # [c, q, n, ig, g]
        return np.ascontiguousarray(
            arr.reshape(NCORES, 128, NIG * G).astype(bf16)
        )

    naA = pack_side(0)
    nbA = pack_side(1)

    mkc = mk.reshape(NCORES, G, TWO, NN)             # [c, g, t, n]
    mA = mkc[:, :, 0].transpose(0, 2, 1)             # [c, n, g]
    mB = mkc[:, :, 1].transpose(0, 2, 1)
    maskA = np.ascontiguousarray(
        np.broadcast_to(mA[:, None], (NCORES, 4, NN, G)).reshape(NCORES, 128, G).astype(bf16)
    )
    maskB = np.ascontiguousarray(
        np.broadcast_to(mB[:, None], (NCORES, 4, NN, G)).reshape(NCORES, 128, G).astype(bf16)
    )
    selfT = np.ascontiguousarray(
        sv.reshape(NCORES, G, D).transpose(0, 2, 1).astype(bf16)
    )  # [c, d, g]

    # shared weights
    W1a = np.ascontiguousarray(Wf[:, :D].T.astype(bf16))          # [d, o]
    w2 = Wf[:, D:].reshape(OUT, D, D)                             # [o, i, j]
    # W2A[p, (ig, r, o)] = w2[o, 4*ig + p//32, (p + 32*r) % 128]
    w2t = np.ascontiguousarray(w2.transpose(1, 2, 0))             # [i, j, o]
    p = np.arange(128)
    q = p // 32
    ig = np.arange(NIG)
    r = np.arange(4)
    i_full = 4 * ig[None, :] + q[:, None]                         # [p, ig]
    j_idx = (p[:, None] + 32 * r[None, :]) % 128                  # [p, r]
    W2A = w2t[i_full[:, :, None], j_idx[:, None, :]]              # [p, ig, r, o]
    W2A = np.ascontiguousarray(W2A.reshape(128, NIG * 4 * OUT).astype(bf16))

    BIG = np.zeros((128, 252), f32)
    rr = np.arange(128)
    BIG[rr, 124 + rr // 32] = 1.0 / 32.0
    BIG = BIG.astype(bf16)
    BD = np.zeros((128, 128), f32)
    BD[rr[:, None] // 32 == rr[None, :] // 32] = 1.0 / 32.0
    BD = BD.astype(bf16)
    # PROT[c, (r-1)*128 + p] = 1 iff c == (p + 32*r) % 128  (rotation matmuls)
    PROT = np.zeros((128, 3 * 128), f32)
    for r_ in range(1, 4):
        pp = np.arange(128)
        PROT[(pp + 32 * r_) % 128, (r_ - 1) * 128 + pp] = 1.0
    PROT = PROT.astype(bf16)

    in_maps = []
    for c in range(NCORES):
        in_maps.append(
            {
                "naA": naA[c],
                "nbA": nbA[c],
                "maskA": maskA[c],
                "maskB": maskB[c],
                "selfT": selfT[c],
                "W1a": W1a,
                "W2A": W2A,
                "BIG": BIG,
                "BD": BD,
                "PROT": PROT,
            }
        )
    return in_maps


def kernel(self_vectors, neighbor_vectors, masks, W, b):
    from concourse.bass_utils import run_bass_kernel_spmd

    if "nc" not in _CACHE:
        _CACHE["nc"] = _build_nc()
    nc = _CACHE["nc"]
    in_maps = _host_prep(self_vectors, neighbor_vectors, masks, W)
    results = run_bass_kernel_spmd(nc, in_maps, list(range(NCORES))).results
    out = np.empty((B, M, OUT), np.float32)
    for c in range(NCORES):
        out[c * BC : (c + 1) * BC] = (
            results[c]["outT"].T.reshape(BC, M, OUT)
        )
    out += np.asarray(b, np.float32)[None, None, :]
    return out


# revision 48
# speedup vs baseline: 1.1701x; 1.1701x over previous
"""Trainium2 Bass kernel for nn_CrossAggregator (gnn_message_passing).

out[g,o] = self[g]·W1[o,:] + ea_g^T A_o eb_g,  g=(b,m), A_o = W[o,128:].reshape(128,128)
ea/eb = masked means over 32 neighbors (t=0 / t=1).

Design v3 (per core, batch/8 data-parallel, G=512 rows), all heavy data bf16:
- single DMA queue (sync), strict priority order: consts, nb, na, W2 —
  chunks land in-place in full-resident tiles (no pool-recycle stalls).
- eb-side: masked-mean via 32 bf16 matmuls with a banded selector (BIG) as
  stationary -> ebT [j,g] in PSUM; Act-copied to SBUF bf16; 3 partition
  rotations ebT[(p+32r)%128, g] via PE permutation matmuls + Act copies
  -> ebTall [128, 4G].
- ea-side: ONE matmul per slab with a block-diagonal ones/32 stationary (BD)
  broadcasts all 4 i's of the slab into 4 partition bands -> repQ [128,G]
  (32 rep passes instead of 128).
- pt: repQ PSUM -> Act copy to bf16 -> one DVE 2x multiply per slab against
  the 4 ebT rotations (in0 dense, in1 stride-0 broadcast) -> pth [128, 4G].
- main contraction: 4 matmuls per slab, stationary W2stat[ig,r][p,o] =
  W2[o, 4ig+p//32, (p+32r)%128]; the rotation is folded into host W2
  packing so all (i,j) pairs are covered exactly once.
  PE total: 32 eb + 3 rot + 32 rep + 128 main + 1 W1 = 196 passes.
- host does only layout transforms (shard/permute/pack/bf16 cast) + out
  transpose + bias add.
"""
import sys
import numpy as np

for _p in ("/opt/trn_rl_repo", "/root/.axon_site/_ro/trn_rl_repo"):
    if _p not in sys.path:
        sys.path.insert(0, _p)

B, M, TWO, NN, D = 1024, 4, 2, 32, 128
OUT = 128
NCORES = 8
BC = B // NCORES          # batches per core
G = BC * M                # 512 rows per core
NIG = D // 4              # 32 slabs of 4 features (partition packing (q,n))
CH = 8                    # slabs per DMA chunk -> 4 chunks per side
NCHUNK = NIG // CH

_CACHE = {}


def _build_nc():
    import os
    import concourse.bacc as bacc_mod
    import concourse.mybir as mybir
    from concourse.tile import TileContext

    F32 = mybir.dt.float32
    BF16 = mybir.dt.bfloat16
    MUL = mybir.AluOpType.mult

    nc = bacc_mod.Bacc(None)

    d_naA = nc.declare_dram_parameter("naA", [128, NIG * G], BF16, isOutput=False)
    d_nbA = nc.declare_dram_parameter("nbA", [128, NIG * G], BF16, isOutput=False)
    d_maskA = nc.declare_dram_parameter("maskA", [128, G], BF16, isOutput=False)
    d_maskB = nc.declare_dram_parameter("maskB", [128, G], BF16, isOutput=False)
    d_selfT = nc.declare_dram_parameter("selfT", [D, G], BF16, isOutput=False)
    d_W1 = nc.declare_dram_parameter("W1a", [D, OUT], BF16, isOutput=False)
    d_W2 = nc.declare_dram_parameter("W2A", [D, NIG * 4 * OUT], BF16, isOutput=False)
    d_BIG = nc.declare_dram_parameter("BIG", [128, 252], BF16, isOutput=False)
    d_BD = nc.declare_dram_parameter("BD", [128, 128], BF16, isOutput=False)
    d_PROT = nc.declare_dram_parameter("PROT", [128, 3 * 128], BF16, isOutput=False)
    d_out = nc.declare_dram_parameter("outT", [OUT, G], F32, isOutput=True)

    NDVE = int(os.environ.get("PT_NDVE", "4"))
    LOOK = int(os.environ.get("REP_LOOK", "2"))
    REP_BUFS = int(os.environ.get("REP_BUFS", "4"))
    EBT_BUFS = int(os.environ.get("EBT_BUFS", "2"))

    with TileContext(nc) as tc:
        with (
            tc.tile_pool(name="const", bufs=1) as cpool,
            tc.tile_pool(name="big", bufs=1) as bigpool,
            tc.tile_pool(name="rq", bufs=3) as rqpool,
            tc.tile_pool(name="pt", bufs=3) as ptpool,
            tc.tile_pool(name="misc", bufs=1) as mpool,
            tc.tile_pool(name="ps_ebt", bufs=EBT_BUFS, space="PSUM") as ps_ebt,
            tc.tile_pool(name="ps_rep", bufs=REP_BUFS, space="PSUM") as ps_rep,
            tc.tile_pool(name="ps_out", bufs=1, space="PSUM") as ps_out,
        ):
            # maskB gates the very first mb mask multiply; everything else
            # can trail the first nb chunk (each DMA issue costs ~0.6us on
            # the sync engine, so nb0 goes as early as possible).
            maskB_t = cpool.tile([128, G], BF16, tag="mb")
            nc.sync.dma_start(out=maskB_t[:], in_=d_maskB[:])

            # full-resident buffers; nb/na land in-place, masks applied
            # in-place (read-before-write on the streaming DVE is safe)
            mb_full = bigpool.tile([128, NIG * G], BF16, tag="mbF")
            ma_full = bigpool.tile([128, NIG * G], BF16, tag="maF")
            w2_full = bigpool.tile([128, NIG * 4 * OUT], BF16, tag="w2F")
            # ebTall holds the 4 rotations twice ([0:4G] and [4G:8G]) so a
            # paired (2-slab) pth op can read a fully dense in0
            ebTall = bigpool.tile([128, 8 * G], BF16, tag="ebAll")

            # heavy DMAs in priority order on the sync queue: nb first
            # (gates the serial eb chain), tapered chunks so the last slabs
            # land with minimal tail; then na0, the rest of the consts,
            # W2 chunk 0 (gates first main matmuls), then the rest.
            NB_CHUNKS = [(0, 2), (2, 6), (8, 4), (12, 4), (16, 4), (20, 4),
                         (24, 4), (28, 2), (30, 1), (31, 1)]
            big_t = cpool.tile([128, 252], BF16, tag="big")
            prot_t = cpool.tile([128, 3 * 128], BF16, tag="prot")
            maskA_t = cpool.tile([128, G], BF16, tag="ma")
            bd_t = cpool.tile([128, 128], BF16, tag="bd")

            def na_dma(s0, sl):
                nc.sync.dma_start(
                    out=ma_full[:, s0 * G : (s0 + sl) * G],
                    in_=d_naA[:, s0 * G : (s0 + sl) * G],
                )

            for ci, (s0, sl) in enumerate(NB_CHUNKS):
                nc.sync.dma_start(
                    out=mb_full[:, s0 * G : (s0 + sl) * G],
                    in_=d_nbA[:, s0 * G : (s0 + sl) * G],
                )
                if ci == 0:
                    # BIG gates the first eb matmul (~1us after mb0 lands)
                    nc.sync.dma_start(out=big_t[:], in_=d_BIG[:])
                elif ci == 1:
                    # PROT gates the rotation matmuls (~12us later)
                    nc.sync.dma_start(out=prot_t[:], in_=d_PROT[:])
            def w2_dma(ig0, nig):
                nc.sync.dma_start(
                    out=w2_full[:, ig0 * 4 * OUT : (ig0 + nig) * 4 * OUT],
                    in_=d_W2[:, ig0 * 4 * OUT : (ig0 + nig) * 4 * OUT],
                )

            nc.sync.dma_start(out=maskA_t[:], in_=d_maskA[:])
            nc.sync.dma_start(out=bd_t[:], in_=d_BD[:])
            na_dma(0, 4)
            w2_dma(0, 2)
            na_dma(4, 4)
            selfT_t = cpool.tile([D, G], BF16, tag="sT")
            nc.sync.dma_start(out=selfT_t[:], in_=d_selfT[:])
            w1_t = cpool.tile([D, OUT], BF16, tag="w1")
            nc.sync.dma_start(out=w1_t[:], in_=d_W1[:])
            # fine-grained W2/na interleave in consumption order: pair k
            # needs W2[2k:2k+2] at ~2.8us intervals and na slabs ~2 pairs
            # ahead — demand ~186 GB/s, so the schedule still meets its
            # deadlines when HBM contention cuts the stream rate.
            w2_dma(2, 2)
            na_dma(8, 4)
            w2_dma(4, 2)
            na_dma(12, 4)
            w2_dma(6, 2)
            na_dma(16, 4)
            w2_dma(8, 2)
            na_dma(20, 4)
            w2_dma(10, 2)
            na_dma(24, 4)
            w2_dma(12, 2)
            na_dma(28, 4)
            w2_dma(14, 2)
            w2_dma(16, 4)
            w2_dma(20, 4)
            w2_dma(24, 4)
            w2_dma(28, 4)

            def emit_mask(full_t, mask_t, s0, sl, eng=None):
                (eng or nc.vector).tensor_tensor(
                    out=full_t[:, s0 * G : (s0 + sl) * G].rearrange(
                        "p (s c) -> p s c", s=sl
                    ),
                    in0=full_t[:, s0 * G : (s0 + sl) * G].rearrange(
                        "p (s c) -> p s c", s=sl
                    ),
                    in1=mask_t[:][:, None, :].broadcast_to([128, sl, G]),
                    op=MUL,
                )

            # ---- EB phase: ebT[j, g] in PSUM via banded-selector matmuls ----
            p_ebt = ps_ebt.tile([128, G], F32, tag="ebt")
            for ci, (s0, sl) in enumerate(NB_CHUNKS):
                emit_mask(mb_full, maskB_t, s0, sl)
                for u in range(sl):
                    jg = s0 + u
                    nc.tensor.matmul(
                        p_ebt[:],
                        big_t[:, 124 - 4 * jg : 252 - 4 * jg],
                        mb_full[:, jg * G : (jg + 1) * G],
                        start=(jg == 0),
                        stop=(jg == NIG - 1),
                    )


            # ma[0:4] right behind the mb-mask tail (na[0:4] lands just
            # after the nb stream); gates reps 0-3
            emit_mask(ma_full, maskA_t, 0, 4)

            # ebT -> SBUF bf16 (rotation 0); rotations 1-3 via PE permutation
            # matmuls (PE is otherwise idle here) + Act copies; then one DVE
            # 4x copy duplicates [0:4G] -> [4G:8G] for the paired pth in0.
            nc.scalar.copy(out=ebTall[:, 0:G], in_=p_ebt[:])
            for r in range(1, 4):
                p_rot = ps_ebt.tile([128, G], F32, tag="ebt")
                nc.tensor.matmul(
                    p_rot[:],
                    prot_t[:, (r - 1) * 128 : r * 128],
                    ebTall[:, 0:G],
                    start=True,
                    stop=True,
                )
                nc.scalar.copy(
                    out=ebTall[:, r * G : (r + 1) * G], in_=p_rot[:]
                )


            # ---- MAIN phase ----
            p_out = ps_out.tile([OUT, G], F32, tag="out")
            nc.tensor.matmul(p_out[:], w1_t[:], selfT_t[:], start=True, stop=False)

            rep_tiles = {}
            rq_tiles = {}
            pth_tiles = {}
            NPAIR = NIG // 2

            def emit_rep(ig):
                rep = ps_rep.tile([128, G], F32, tag="rep")
                nc.tensor.matmul(
                    rep[:],
                    bd_t[:],
                    ma_full[:, ig * G : (ig + 1) * G],
                    start=True,
                    stop=True,
                )
                rep_tiles[ig] = rep

            def emit_cp(k):
                # rq pair tile: halves written by two Act copies
                rq = rqpool.tile([128, 2 * G], BF16, tag="rq")
                for u in range(2):
                    rep = rep_tiles.pop(2 * k + u)
                    nc.scalar.copy(out=rq[:, u * G : (u + 1) * G], in_=rep[:])
                rq_tiles[k] = rq

            def emit_pth(k, split=False):
                # one DVE op per slab PAIR: out/in0 fully dense 4D, in1
                # broadcasts each slab's rq across the 4 rotations.
                # split=True: two 4G ops reading ebTall[0:4G] only — used
                # for pair 0 (before the dup lands) and the last pair
                # (first 4 main matmuls start half an op earlier).
                rq = rq_tiles.pop(k)
                pth = ptpool.tile([128, 8 * G], BF16, tag="pth")
                if split:
                    for u in range(2):
                        nc.vector.tensor_tensor(
                            out=pth[:, u * 4 * G : (u + 1) * 4 * G].rearrange(
                                "p (r c) -> p r c", r=4
                            ),
                            in0=ebTall[:, 0 : 4 * G].rearrange(
                                "p (r c) -> p r c", r=4
                            ),
                            in1=rq[:, u * G : (u + 1) * G][:, None, :]
                            .broadcast_to([128, 4, G]),
                            op=MUL,
                        )
                else:
                    nc.vector.tensor_tensor(
                        out=pth[:].rearrange("p (i r c) -> p i r c", i=2, r=4),
                        in0=ebTall[:].rearrange("p (i r c) -> p i r c", i=2, r=4),
                        in1=rq[:].rearrange("p (i c) -> p i c", i=2)[:, :, None, :]
                        .broadcast_to([128, 2, 4, G]),
                        op=MUL,
                    )
                pth_tiles[k] = pth

            for ig in range(2 * min(LOOK, NPAIR)):
                emit_rep(ig)
            emit_cp(0)
            emit_pth(0, split=True)
            # duplicate ebTall[0:4G] -> [4G:8G] for the paired in0 (4x DVE
            # copy, off the pth0 critical path)
            nc.vector.tensor_scalar_mul(
                ebTall[:, 4 * G : 8 * G], ebTall[:, 0 : 4 * G], 1.0
            )
            # remaining na masks threaded through the pair loop in 8/4-slab
            # pieces, each well ahead of its rep deadline (rep(2k+2*LOOK+1)
            # is emitted at pair k).
            MA_SCHED = {0: (4, 4), 1: (8, 8), 2: (16, 8), 4: (24, 8)}
            for k in range(NPAIR):
                if k in MA_SCHED:
                    s0, sl = MA_SCHED[k]
                    emit_mask(ma_full, maskA_t, s0, sl)
                if k + LOOK < NPAIR:
                    emit_rep(2 * k + 2 * LOOK)
                    emit_rep(2 * k + 2 * LOOK + 1)
                if k + 1 < NPAIR:
                    emit_cp(k + 1)
                    emit_pth(k + 1, split=(k + 1 == NPAIR - 1))
                pth = pth_tiles.pop(k)
                for u in range(2):
                    ig = 2 * k + u
                    for r in range(4):
                        nc.tensor.matmul(
                            p_out[:],
                            w2_full[:, (ig * 4 + r) * OUT : (ig * 4 + r + 1) * OUT],
                            pth[:, (u * 4 + r) * G : (u * 4 + r + 1) * G],
                            start=False,
                            stop=(ig == NIG - 1 and r == 3),
                        )

            out_sb = mpool.tile([OUT, G], F32, tag="osb")
            nc.scalar.copy(out=out_sb[:, 0 : G // 2], in_=p_out[:, 0 : G // 2])
            nc.sync.dma_start(out=d_out[:, 0 : G // 2], in_=out_sb[:, 0 : G // 2])
            nc.scalar.copy(out=out_sb[:, G // 2 :], in_=p_out[:, G // 2 :])
            nc.sync.dma_start(out=d_out[:, G // 2 :], in_=out_sb[:, G // 2 :])

    nc.finalize()
    return nc


def _host_prep(self_vectors, neighbor_vectors, masks, W):
    import ml_dtypes

    f32 = np.float32
    bf16 = ml_dtypes.bfloat16
    sv = np.asarray(self_vectors, dtype=f32)
    nv = np.asarray(neighbor_vectors, dtype=f32)
    mk = np.asarray(masks, dtype=f32)
    Wf = np.asarray(W, dtype=f32)

    # per-core packs: partition p = (q, n) holds feature j = 4*ig + q
    # cols = (ig, g)
    nvc = nv.reshape(NCORES, G, TWO, NN, D)          # [c, g, t, n, d]

    def pack_side(t):
        arr = nvc[:, :, t]                            # [c, g, n, d]
        arr = arr.transpose(0, 3, 2, 1)               # [c, d, n, g]
        arr = arr.reshape(NCORES, NIG, 4, NN, G)      # [c, ig, q, n, g]
        arr = arr.transpose(0, 2, 3, 1, 4)            # [c, q, n, ig, g]
        return np.ascontiguousarray(
            arr.reshape(NCORES, 128, NIG * G).astype(bf16)
        )

    naA = pack_side(0)
    nbA = pack_side(1)

    mkc = mk.reshape(NCORES, G, TWO, NN)             # [c, g, t, n]
    mA = mkc[:, :, 0].transpose(0, 2, 1)             # [c, n, g]
    mB = mkc[:, :, 1].transpose(0, 2, 1)
    maskA = np.ascontiguousarray(
        np.broadcast_to(mA[:, None], (NCORES, 4, NN, G)).reshape(NCORES, 128, G).astype(bf16)
    )
    maskB = np.ascontiguousarray(
        np.broadcast_to(mB[:, None], (NCORES, 4, NN, G)).reshape(NCORES, 128, G).astype(bf16)
    )
    selfT = np.ascontiguousarray(
        sv.reshape(NCORES, G, D).transpose(0, 2, 1).astype(bf16)
    )  # [c, d, g]

    # shared weights
    W1a = np.ascontiguousarray(Wf[:, :D].T.astype(bf16))          # [d, o]
    w2 = Wf[:, D:].reshape(OUT, D, D)                             # [o, i, j]
    # W2A[p, (ig, r, o)] = w2[o, 4*ig + p//32, (p + 32*r) % 128]
    w2t = np.ascontiguousarray(w2.transpose(1, 2, 0))             # [i, j, o]
    p = np.arange(128)
    q = p // 32
    ig = np.arange(NIG)
    r = np.arange(4)
    i_full = 4 * ig[None, :] + q[:, None]                         # [p, ig]
    j_idx = (p[:, None] + 32 * r[None, :]) % 128                  # [p, r]
    W2A = w2t[i_full[:, :, None], j_idx[:, None, :]]              # [p, ig, r, o]
    W2A = np.ascontiguousarray(W2A.reshape(128, NIG * 4 * OUT).astype(bf16))

    BIG = np.zeros((128, 252), f32)
    rr = np.arange(128)
    BIG[rr, 124 + rr // 32] = 1.0 / 32.0
    BIG = BIG.astype(bf16)
    BD = np.zeros((128, 128), f32)
    BD[rr[:, None] // 32 == rr[None, :] // 32] = 1.0 / 32.0
    BD = BD.astype(bf16)
    # PROT[c, (r-1)*128 + p] = 1 iff c == (p + 32*r) % 128  (rotation matmuls)
    PROT = np.zeros((128, 3 * 128), f32)
    for r_ in range(1, 4):
        pp = np.arange(128)
        PROT[(pp + 32 * r_) % 128, (r_ - 1) * 128 + pp] = 1.0
    PROT = PROT.astype(bf16)

    in_maps = []
    for c in range(NCORES):
        in_maps.append(
            {
                "naA": naA[c],
                "nbA": nbA[c],
                "maskA": maskA[c],
                "maskB": maskB[c],
                "selfT": selfT[c],
                "W1a": W1a,
                "W2A": W2A,
                "BIG": BIG,
                "BD": BD,
                "PROT": PROT,
            }
        )
    return in_maps


def kernel(self_vectors, neighbor_vectors, masks, W, b):
    from concourse.bass_utils import run_bass_kernel_spmd

    if "nc" not in _CACHE:
        _CACHE["nc"] = _build_nc()
    nc = _CACHE["nc"]
    in_maps = _host_prep(self_vectors, neighbor_vectors, masks, W)
    results = run_bass_kernel_spmd(nc, in_maps, list(range(NCORES))).results
    out = np.empty((B, M, OUT), np.float32)
    for c in range(NCORES):
        out[c * BC : (c + 1) * BC] = (
            results[c]["outT"].T.reshape(BC, M, OUT)
        )
    out += np.asarray(b, np.float32)[None, None, :]
    return out
